# revision 13
# baseline (speedup 1.0000x reference)
"""MDRNN2D (4-direction 2D GRU) Trainium2 Bass kernel — bf16 wavefront v2.

Sharding: 8 cores = 4 scan directions x 2 batch halves (16 each).
Each core runs a wavefront over the 125 anti-diagonals of its (flipped)
63x63 grid. Hidden state for a diagonal lives in SBUF as (128=hid,
cells*16) bf16; h_up / h_left of diag t are 16-col-shifted slices of
diag t-1's buffer (3-buffer ring).

Each diagonal is split at a FIXED global cell boundary (cell 32) into
chunks A (i<32) and B (i>=32).  Cell neighbors are always on the
previous diagonal, so A(t) and B(t) both depend only on round t-1 =>
the two chunk-streams are independent chains that pipeline against
each other on the engines.

v2 structural changes vs v1:
- All biases folded into the matmuls via a ones-row on x (K=65) and the
  bias row appended to Wx; z-gate weights pre-negated on the host so
  z' = sigmoid(pz) directly.  No ACT bias ports used.
- pr and pz share one PSUM tile (prz, [128, 2w]); on narrow chunks one
  merged sigmoid ACT produces r|z' in a single instruction.
- Wide chunks: the n-gate add (gx_n + r*png) is done by accumulating
  an identity matmul of v = r*png into the open pnx PSUM group; tanh
  then reads PSUM directly.  Kills the 1x-throughput PSUM-source DVE
  add.  Narrow chunks keep the DVE add (fewer cross-engine hops).
- Narrow chunks use fused scalar_tensor_tensor for the tail:
  mu = 0.5*s - n; g2 = z'*mu; h = 0.5*s - g2  (4 DVE ops vs 5).
- No dummy warm-up matmuls: real work (I*v) keeps PE's HAM warm.
- Wider chunk emitted first each round (fixes descending-side stall).
"""

import os

import numpy as np

B, IN, H_IMG, W_IMG, HID = 32, 64, 64, 64, 128
INB = IN + 1      # x rows + ones-row (bias fold)
G = 63            # computed grid is (H-1, W-1)
ND = 2 * G - 1    # number of anti-diagonals
SB = 16           # batch per core
STOT = G * G * SB
TOTAL = STOT
GB = 32           # fixed A/B chunk boundary in global cell index
SMALL_W = 256     # chunks narrower than this use the low-latency path

# (i0, i1, ncells, col_offset) per diagonal; cells of diag t are (i, t-i),
# i in [i0, i1], stored as SB consecutive columns per cell, i ascending.
_DIAG = []
_off = 0
for _t in range(ND):
    _i0, _i1 = max(0, _t - (G - 1)), min(_t, G - 1)
    _n = _i1 - _i0 + 1
    _DIAG.append((_i0, _i1, _n, _off))
    _off += _n * SB
assert _off == STOT

_FLIPS = [(False, False), (True, False), (False, True), (True, True)]

_PROG_CACHE = {}


def _build_program():
    import concourse.mybir as mybir
    import concourse.tile as tile
    from concourse import bacc

    f32 = mybir.dt.float32
    bf16 = mybir.dt.bfloat16
    AF = mybir.ActivationFunctionType
    OP = mybir.AluOpType

    nc = bacc.Bacc()
    xd = nc.declare_dram_parameter("xd", [INB, TOTAL], bf16, isOutput=False)
    wxp = nc.declare_dram_parameter("wx", [INB, 3 * HID], bf16, isOutput=False)
    whp = nc.declare_dram_parameter("wh", [HID, 3 * HID], bf16, isOutput=False)
    wh2p = nc.declare_dram_parameter("wh2", [HID, 3 * HID], bf16, isOutput=False)
    idp = nc.declare_dram_parameter("ident", [HID, HID], bf16, isOutput=False)
    od = nc.declare_dram_parameter("od", [HID, TOTAL], bf16, isOutput=True)

    with tile.TileContext(nc) as tc:
        with (
            tc.tile_pool(name="const", bufs=1) as cpool,
            tc.tile_pool(name="hbuf", bufs=1) as hpool,
            tc.tile_pool(name="xin", bufs=6) as xpool,
            tc.tile_pool(name="ps", bufs=2, space="PSUM") as ppool,
            tc.tile_pool(name="work", bufs=3) as wpool,
        ):
            wx_t = cpool.tile([INB, 3 * HID], bf16, tag="wx")
            wh_t = cpool.tile([HID, 3 * HID], bf16, tag="wh")
            wh2_t = cpool.tile([HID, 3 * HID], bf16, tag="wh2")
            id_t = cpool.tile([HID, HID], bf16, tag="ident")
            nc.sync.dma_start(out=wx_t[:], in_=wxp[:])
            nc.sync.dma_start(out=wh_t[:], in_=whp[:])
            nc.sync.dma_start(out=wh2_t[:], in_=wh2p[:])
            nc.sync.dma_start(out=id_t[:], in_=idp[:])

            hbufs = []
            for k in range(3):
                hb = hpool.tile([HID, (G + 2) * SB], bf16, tag=f"h{k}",
                                name=f"hring{k}")
                hbufs.append(hb)
                nc.vector.memset(hb[:], 0.0)

            wxg = [wx_t[:, g * HID:(g + 1) * HID] for g in range(3)]
            whg = [wh_t[:, g * HID:(g + 1) * HID] for g in range(3)]
            wh2g = [wh2_t[:, g * HID:(g + 1) * HID] for g in range(3)]

            x_tiles = {}

            def parts_of(t):
                i0, i1, n, _ = _DIAG[t]
                if i1 < GB:
                    return [(0, n)]
                if i0 >= GB:
                    return [(0, n)]
                return [(0, GB - i0), (GB - i0, n)]

            # Per-chunk state dicts built in phases so each engine's FIFO
            # order can interleave the chunks of one round.
            def mk(t, c0, c1):
                i0, i1, n, off = _DIAG[t]
                w = (c1 - c0) * SB
                st = {
                    "t": t, "c0": c0, "c1": c1, "w": w,
                    "small": w < SMALL_W,
                    "cur": hbufs[t % 3], "prev": hbufs[(t - 1) % 3],
                    "xs": c0 * SB,
                    "ob": (i0 + 1 + c0) * SB,
                    "up": (i0 + c0) * SB,
                    "lf": (i0 + 1 + c0) * SB,
                }
                st["prz"] = ppool.tile([HID, 2, 512], f32, tag="prz", name="prz")
                st["png"] = ppool.tile([HID, w], f32, tag="png", name="png")
                st["pnx"] = ppool.tile([HID, w], f32, tag="pnx", name="pnx")
                return st

            def pe_x(st):
                # x-part matmuls (incl. ones-row bias): open all 3 groups
                w, xs = st["w"], st["xs"]
                xin = x_tiles[st["t"]][:, xs:xs + w]
                prz, pnx = st["prz"], st["pnx"]
                # wide path: pnx group stays open for the I*v accumulate
                nc.tensor.matmul(pnx[:], wxg[2], xin, start=True,
                                 stop=st["small"])
                nc.tensor.matmul(prz[:, 0, 0:w], wxg[0], xin, start=True, stop=False)
                nc.tensor.matmul(prz[:, 1, 0:w], wxg[1], xin, start=True, stop=False)

            def pe_h(st, gates):
                w = st["w"]
                prev = st["prev"]
                h_up = prev[:, st["up"]:st["up"] + w]
                h_left = prev[:, st["lf"]:st["lf"] + w]
                prz, png = st["prz"], st["png"]
                for gsel in gates:
                    if gsel == "r":
                        nc.tensor.matmul(prz[:, 0, 0:w], whg[0], h_up,
                                         start=False, stop=False)
                        nc.tensor.matmul(prz[:, 0, 0:w], wh2g[0], h_left,
                                         start=False, stop=True)
                    elif gsel == "n":
                        nc.tensor.matmul(png[:], whg[2], h_up,
                                         start=True, stop=False)
                        nc.tensor.matmul(png[:], wh2g[2], h_left,
                                         start=False, stop=True)
                    else:
                        nc.tensor.matmul(prz[:, 1, 0:w], whg[1], h_up,
                                         start=False, stop=False)
                        nc.tensor.matmul(prz[:, 1, 0:w], wh2g[1], h_left,
                                         start=False, stop=True)

            def pe_fill(st, k):
                # HAM-warmth filler: the PE clock-gate re-throttles to 1.2GHz
                # whenever its activity duty cycle drops, so the engine-idle
                # windows during each round's elementwise tail must be packed
                # with throwaway matmuls (into an already-consumed psum bank).
                for _ in range(k):
                    nc.tensor.matmul(st["prz"][:, 0, 0:512], id_t[:],
                                     st["prev"][:, 0:512], start=True,
                                     stop=True, skip_group_check=True)

            def pe_iv(st):
                # accumulate I*v into the still-open pnx group (wide path)
                nc.tensor.matmul(st["pnx"][:], id_t[:], st["v"][:],
                                 start=False, stop=True)

            def act_rz(st):
                w = st["w"]
                r_t = wpool.tile([HID, w], bf16, tag="r", name="r")
                nc.scalar.activation(r_t[:], st["prz"][:, 0, 0:w], AF.Sigmoid)
                st["r"] = r_t[:]
                zp_t = wpool.tile([HID, w], bf16, tag="zp", name="zp")
                nc.scalar.activation(zp_t[:], st["prz"][:, 1, 0:w],
                                     AF.Sigmoid)
                st["zp"] = zp_t[:]

            def act_n(st):
                w = st["w"]
                n_t = wpool.tile([HID, w], bf16, tag="n", name="n")
                if st["small"]:
                    nc.scalar.activation(n_t[:], st["wt"][:], AF.Tanh)
                else:
                    nc.scalar.activation(n_t[:], st["pnx"][:], AF.Tanh)
                st["n"] = n_t

            def dve_prep(st):
                # s = h_up + h_left (t = 0.5*s only on the wide path)
                w = st["w"]
                prev = st["prev"]
                h_up = prev[:, st["up"]:st["up"] + w]
                h_left = prev[:, st["lf"]:st["lf"] + w]
                s_t = wpool.tile([HID, w], bf16, tag="s", name="s")
                nc.gpsimd.tensor_add(s_t[:], h_up, h_left)
                st["s"] = s_t
                t_t = wpool.tile([HID, w], bf16, tag="tt", name="tt")
                nc.gpsimd.tensor_scalar_mul(t_t[:], s_t[:], 0.5)
                st["tt"] = t_t

            def dve_v(st):
                w = st["w"]
                v_t = wpool.tile([HID, w], bf16, tag="v", name="v")
                nc.vector.tensor_mul(v_t[:], st["r"], st["png"][:])
                st["v"] = v_t
                if st["small"]:
                    wt = wpool.tile([HID, w], bf16, tag="wt", name="wt")
                    nc.vector.tensor_add(wt[:], v_t[:], st["pnx"][:])
                    st["wt"] = wt

            def dve_tail(st):
                w = st["w"]
                cur, ob = st["cur"], st["ob"]
                n_t, t_t = st["n"], st["tt"]
                u_t = wpool.tile([HID, w], bf16, tag="u", name="u")
                nc.vector.tensor_sub(u_t[:], n_t[:], t_t[:])
                g_t = wpool.tile([HID, w], bf16, tag="g", name="g")
                nc.vector.tensor_mul(g_t[:], st["zp"], u_t[:])
                nc.vector.tensor_add(cur[:, ob:ob + w], g_t[:], t_t[:])

            for t in range(ND):
                i0, i1, n, off = _DIAG[t]
                x_t = xpool.tile([INB, n * SB], bf16, tag="xt")
                nc.sync.dma_start(out=x_t[:], in_=xd[:, off:off + n * SB])
                x_tiles[t] = x_t

                chunks = [mk(t, c0, c1) for c0, c1 in parts_of(t)]
                # wider chunk first: its chain is the round's long pole
                chunks.sort(key=lambda s: -s["w"])
                c1s = chunks[0]
                c2s = chunks[1] if len(chunks) > 1 else None

                # Emission order follows dataflow; per-engine FIFO order is
                # what each engine sees:
                #  PE:  x(c1) x(c2) | h(c1) | h(c2,rn) I*v(c1) h(c2,z) I*v(c2)
                #  ACT: rz(c1) rz(c2) n(c1) n(c2)
                #  DVE: prep(c1) prep(c2) v(c1) tail(c1) v(c2) tail(c2)
                wtot = n * SB
                ch = 2 if len(chunks) > 1 else 1
                wmax = c1s["w"]
                pe_real = 4.17 * wtot + 1200.0
                dve_est = 600.0 * ch + 2.71 * wtot
                act_est = 480.0 * ch + 2.5 * wtot
                chain_est = 1500.0 + 5.52 * wmax
                round_est = max(dve_est, act_est, chain_est, pe_real)
                ndum = max(0, min(10, int((round_est - pe_real) / 213)))
                k1 = min(5, ndum) if ch == 2 else min(2, ndum)
                pe_x(c1s)
                if c2s is not None:
                    pe_x(c2s)
                pe_h(c1s, ("r", "n", "z"))
                if c2s is not None:
                    pe_h(c2s, ("r",))
                dve_prep(c1s)
                if c2s is not None:
                    dve_prep(c2s)
                act_rz(c1s)
                dve_v(c1s)
                if c2s is None and not c1s["small"]:
                    pe_fill(c1s, min(2, ndum))
                if not c1s["small"]:
                    pe_iv(c1s)
                if c2s is not None:
                    pe_h(c2s, ("n", "z"))
                act_n(c1s)
                dve_tail(c1s)
                if c2s is not None:
                    act_rz(c2s)
                    dve_v(c2s)
                    pe_fill(c1s, k1)
                    if not c2s["small"]:
                        pe_iv(c2s)
                    act_n(c2s)
                    dve_tail(c2s)
                    pe_fill(c1s, ndum - k1)
                else:
                    pe_fill(c1s, ndum - min(2, ndum))

                # one output DMA per diagonal, issued from the gpsimd queue
                cur = hbufs[t % 3]
                nc.sync.dma_start(
                    out=od[:, off:off + n * SB],
                    in_=cur[:, (i0 + 1) * SB:(i1 + 2) * SB])

    nc.finalize()
    return nc


def _host_prep(x, Wx, Wh, Wh2, b):
    """Build per-core input maps (8 cores = 4 dirs x 2 batch halves)."""
    import ml_dtypes
    bf16 = ml_dtypes.bfloat16
    xr = np.ascontiguousarray(np.transpose(x, (2, 3, 0, 1))[:G, :G])  # (G,G,B,IN)
    ident = np.eye(HID, dtype=np.float32).astype(bf16)
    in_maps = []
    for d, (fy, fx) in enumerate(_FLIPS):
        xg = xr[::-1] if fy else xr
        xg = xg[:, ::-1] if fx else xg
        # fold bias into a ones-row; negate the z-gate block so
        # z' = sigmoid(pz) directly
        wx_e = np.concatenate([Wx[d], b[d][None, :]], axis=0).astype(np.float32)
        wh_e = np.array(Wh[d], np.float32)
        wh2_e = np.array(Wh2[d], np.float32)
        for m in (wx_e, wh_e, wh2_e):
            m[:, HID:2 * HID] *= -1.0
        wx_b = np.ascontiguousarray(wx_e).astype(bf16)
        wh_b = np.ascontiguousarray(wh_e).astype(bf16)
        wh2_b = np.ascontiguousarray(wh2_e).astype(bf16)
        for half in range(2):
            b0 = half * SB
            xh = xg[:, :, b0:b0 + SB]          # (G,G,SB,IN)
            xdiag = np.empty((INB, TOTAL), bf16)
            xdiag[IN, :] = np.float32(1.0)
            for t in range(ND):
                i0, i1, n, off = _DIAG[t]
                ii = np.arange(i0, i1 + 1)
                blk = xh[ii, t - ii]           # (n, SB, IN)
                xdiag[:IN, off:off + n * SB] = blk.reshape(n * SB, IN).T
            in_maps.append({
                "xd": xdiag,
                "wx": wx_b,
                "wh": wh_b,
                "wh2": wh2_b,
                "ident": ident,
            })
    return in_maps


def _host_gather(results):
    out_map = np.ones((4, H_IMG, W_IMG, B, HID), np.float32)
    for d, (fy, fx) in enumerate(_FLIPS):
        o = np.empty((G, G, B, HID), np.float32)
        for half in range(2):
            od = np.asarray(results[d * 2 + half]["od"], np.float32)  # (HID, TOTAL)
            b0 = half * SB
            for t in range(ND):
                i0, i1, n, off = _DIAG[t]
                sl = od[:, off:off + n * SB]
                blk = sl.T.reshape(n, SB, HID)
                ii = np.arange(i0, i1 + 1)
                o[ii, t - ii, b0:b0 + SB] = blk
        o = o[::-1] if fy else o
        o = o[:, ::-1] if fx else o
        oy, ox = (1 if fy else 0), (1 if fx else 0)
        out_map[d, oy:oy + G, ox:ox + G] = o
    return np.ascontiguousarray(np.transpose(out_map, (3, 4, 0, 1, 2)))


def kernel(x, Wx, Wh, Wh2, b):
    from concourse.bass_utils import run_bass_kernel_spmd

    if "prog" not in _PROG_CACHE:
        _PROG_CACHE["prog"] = _build_program()
    nc = _PROG_CACHE["prog"]

    in_maps = _host_prep(
        np.asarray(x, np.float32), np.asarray(Wx, np.float32),
        np.asarray(Wh, np.float32), np.asarray(Wh2, np.float32),
        np.asarray(b, np.float32))
    trace = os.environ.get("MDRNN_TRACE", "0") == "1"
    res = run_bass_kernel_spmd(nc, in_maps, list(range(8)), trace=trace)
    out = _host_gather(res.results)
    if trace:
        kernel.last_exec_time_ns = res.exec_time_ns
        kernel.last_profile = res
    return out


# revision 14
# speedup vs baseline: 1.7375x; 1.7375x over previous
"""MDRNN2D (4-direction 2D GRU) Trainium2 Bass kernel — bf16 wavefront v2.

Sharding: 8 cores = 4 scan directions x 2 batch halves (16 each).
Each core runs a wavefront over the 125 anti-diagonals of its (flipped)
63x63 grid. Hidden state for a diagonal lives in SBUF as (128=hid,
cells*16) bf16; h_up / h_left of diag t are 16-col-shifted slices of
diag t-1's buffer (3-buffer ring).

Each diagonal is split at a FIXED global cell boundary (cell 32) into
chunks A (i<32) and B (i>=32).  Cell neighbors are always on the
previous diagonal, so A(t) and B(t) both depend only on round t-1 =>
the two chunk-streams are independent chains that pipeline against
each other on the engines.

v2 structural changes vs v1:
- All biases folded into the matmuls via a ones-row on x (K=65) and the
  bias row appended to Wx; z-gate weights pre-negated on the host so
  z' = sigmoid(pz) directly.  No ACT bias ports used.
- pr and pz share one PSUM tile (prz, [128, 2w]); on narrow chunks one
  merged sigmoid ACT produces r|z' in a single instruction.
- Wide chunks: the n-gate add (gx_n + r*png) is done by accumulating
  an identity matmul of v = r*png into the open pnx PSUM group; tanh
  then reads PSUM directly.  Kills the 1x-throughput PSUM-source DVE
  add.  Narrow chunks keep the DVE add (fewer cross-engine hops).
- Narrow chunks use fused scalar_tensor_tensor for the tail:
  mu = 0.5*s - n; g2 = z'*mu; h = 0.5*s - g2  (4 DVE ops vs 5).
- No dummy warm-up matmuls: real work (I*v) keeps PE's HAM warm.
- Wider chunk emitted first each round (fixes descending-side stall).
"""

import os

import numpy as np

B, IN, H_IMG, W_IMG, HID = 32, 64, 64, 64, 128
INB = IN + 1      # x rows + ones-row (bias fold)
G = 63            # computed grid is (H-1, W-1)
ND = 2 * G - 1    # number of anti-diagonals
SB = 16           # batch per core
STOT = G * G * SB
TOTAL = STOT
GB = 32           # fixed A/B chunk boundary in global cell index
SMALL_W = 256     # chunks narrower than this use the low-latency path

# (i0, i1, ncells, col_offset) per diagonal; cells of diag t are (i, t-i),
# i in [i0, i1], stored as SB consecutive columns per cell, i ascending.
_DIAG = []
_off = 0
for _t in range(ND):
    _i0, _i1 = max(0, _t - (G - 1)), min(_t, G - 1)
    _n = _i1 - _i0 + 1
    _DIAG.append((_i0, _i1, _n, _off))
    _off += _n * SB
assert _off == STOT

_FLIPS = [(False, False), (True, False), (False, True), (True, True)]

_PROG_CACHE = {}


def _build_program():
    import concourse.mybir as mybir
    import concourse.tile as tile
    from concourse import bacc

    f32 = mybir.dt.float32
    bf16 = mybir.dt.bfloat16
    AF = mybir.ActivationFunctionType
    OP = mybir.AluOpType

    nc = bacc.Bacc()
    xd = nc.declare_dram_parameter("xd", [INB, TOTAL], bf16, isOutput=False)
    wxp = nc.declare_dram_parameter("wx", [INB, 3 * HID], bf16, isOutput=False)
    whp = nc.declare_dram_parameter("wh", [HID, 3 * HID], bf16, isOutput=False)
    wh2p = nc.declare_dram_parameter("wh2", [HID, 3 * HID], bf16, isOutput=False)
    idp = nc.declare_dram_parameter("ident", [HID, HID], bf16, isOutput=False)
    od = nc.declare_dram_parameter("od", [HID, TOTAL], bf16, isOutput=True)

    with tile.TileContext(nc) as tc:
        with (
            tc.tile_pool(name="const", bufs=1) as cpool,
            tc.tile_pool(name="hbuf", bufs=1) as hpool,
            tc.tile_pool(name="xin", bufs=6) as xpool,
            tc.tile_pool(name="ps", bufs=2, space="PSUM") as ppool,
            tc.tile_pool(name="work", bufs=3) as wpool,
        ):
            wx_t = cpool.tile([INB, 3 * HID], bf16, tag="wx")
            wh_t = cpool.tile([HID, 3 * HID], bf16, tag="wh")
            wh2_t = cpool.tile([HID, 3 * HID], bf16, tag="wh2")
            id_t = cpool.tile([HID, HID], bf16, tag="ident")
            nc.sync.dma_start(out=wx_t[:], in_=wxp[:])
            nc.sync.dma_start(out=wh_t[:], in_=whp[:])
            nc.sync.dma_start(out=wh2_t[:], in_=wh2p[:])
            nc.sync.dma_start(out=id_t[:], in_=idp[:])

            hbufs = []
            for k in range(3):
                hb = hpool.tile([HID, (G + 2) * SB], bf16, tag=f"h{k}",
                                name=f"hring{k}")
                hbufs.append(hb)
                nc.vector.memset(hb[:], 0.0)

            wxg = [wx_t[:, g * HID:(g + 1) * HID] for g in range(3)]
            whg = [wh_t[:, g * HID:(g + 1) * HID] for g in range(3)]
            wh2g = [wh2_t[:, g * HID:(g + 1) * HID] for g in range(3)]

            x_tiles = {}

            def parts_of(t):
                i0, i1, n, _ = _DIAG[t]
                if i1 < GB:
                    return [(0, n)]
                if i0 >= GB:
                    return [(0, n)]
                return [(0, GB - i0), (GB - i0, n)]

            # Per-chunk state dicts built in phases so each engine's FIFO
            # order can interleave the chunks of one round.
            def mk(t, c0, c1):
                i0, i1, n, off = _DIAG[t]
                w = (c1 - c0) * SB
                st = {
                    "t": t, "c0": c0, "c1": c1, "w": w,
                    "small": w < SMALL_W,
                    "cur": hbufs[t % 3], "prev": hbufs[(t - 1) % 3],
                    "xs": c0 * SB,
                    "ob": (i0 + 1 + c0) * SB,
                    "up": (i0 + c0) * SB,
                    "lf": (i0 + 1 + c0) * SB,
                }
                st["prz"] = ppool.tile([HID, 2, 512], f32, tag="prz", name="prz")
                st["png"] = ppool.tile([HID, w], f32, tag="png", name="png")
                st["pnx"] = ppool.tile([HID, w], f32, tag="pnx", name="pnx")
                return st

            def pe_x(st):
                # x-part matmuls (incl. ones-row bias): open all 3 groups
                w, xs = st["w"], st["xs"]
                xin = x_tiles[st["t"]][:, xs:xs + w]
                prz, pnx = st["prz"], st["pnx"]
                # wide path: pnx group stays open for the I*v accumulate
                nc.tensor.matmul(pnx[:], wxg[2], xin, start=True,
                                 stop=st["small"])
                nc.tensor.matmul(prz[:, 0, 0:w], wxg[0], xin, start=True, stop=False)
                nc.tensor.matmul(prz[:, 1, 0:w], wxg[1], xin, start=True, stop=False)

            def pe_h(st, gates):
                w = st["w"]
                prev = st["prev"]
                h_up = prev[:, st["up"]:st["up"] + w]
                h_left = prev[:, st["lf"]:st["lf"] + w]
                prz, png = st["prz"], st["png"]
                for gsel in gates:
                    if gsel == "r":
                        nc.tensor.matmul(prz[:, 0, 0:w], whg[0], h_up,
                                         start=False, stop=False)
                        nc.tensor.matmul(prz[:, 0, 0:w], wh2g[0], h_left,
                                         start=False, stop=True)
                    elif gsel == "n":
                        nc.tensor.matmul(png[:], whg[2], h_up,
                                         start=True, stop=False)
                        nc.tensor.matmul(png[:], wh2g[2], h_left,
                                         start=False, stop=True)
                    else:
                        nc.tensor.matmul(prz[:, 1, 0:w], whg[1], h_up,
                                         start=False, stop=False)
                        nc.tensor.matmul(prz[:, 1, 0:w], wh2g[1], h_left,
                                         start=False, stop=True)

            def pe_fill(st, k):
                # HAM-warmth filler: the PE clock-gate re-throttles to 1.2GHz
                # whenever its activity duty cycle drops, so the engine-idle
                # windows during each round's elementwise tail must be packed
                # with throwaway matmuls (into an already-consumed psum bank).
                for _ in range(k):
                    nc.tensor.matmul(st["prz"][:, 0, 0:512], id_t[:],
                                     st["prev"][:, 0:512], start=True,
                                     stop=True, skip_group_check=True)

            def pe_iv(st):
                # accumulate I*v into the still-open pnx group (wide path)
                nc.tensor.matmul(st["pnx"][:], id_t[:], st["v"][:],
                                 start=False, stop=True)

            def act_rz(st):
                w = st["w"]
                r_t = wpool.tile([HID, w], bf16, tag="r", name="r")
                nc.scalar.activation(r_t[:], st["prz"][:, 0, 0:w], AF.Sigmoid)
                st["r"] = r_t[:]
                zp_t = wpool.tile([HID, w], bf16, tag="zp", name="zp")
                nc.scalar.activation(zp_t[:], st["prz"][:, 1, 0:w],
                                     AF.Sigmoid)
                st["zp"] = zp_t[:]

            def act_n(st):
                w = st["w"]
                n_t = wpool.tile([HID, w], bf16, tag="n", name="n")
                if st["small"]:
                    nc.scalar.activation(n_t[:], st["wt"][:], AF.Tanh)
                else:
                    nc.scalar.activation(n_t[:], st["pnx"][:], AF.Tanh)
                st["n"] = n_t

            def dve_prep(st):
                # s = h_up + h_left (t = 0.5*s only on the wide path)
                w = st["w"]
                prev = st["prev"]
                h_up = prev[:, st["up"]:st["up"] + w]
                h_left = prev[:, st["lf"]:st["lf"] + w]
                s_t = wpool.tile([HID, w], bf16, tag="s", name="s")
                nc.vector.tensor_add(s_t[:], h_up, h_left)
                st["s"] = s_t
                t_t = wpool.tile([HID, w], bf16, tag="tt", name="tt")
                nc.vector.tensor_scalar_mul(t_t[:], s_t[:], 0.5)
                st["tt"] = t_t

            def dve_v(st):
                w = st["w"]
                v_t = wpool.tile([HID, w], bf16, tag="v", name="v")
                nc.vector.tensor_mul(v_t[:], st["r"], st["png"][:])
                st["v"] = v_t
                if st["small"]:
                    wt = wpool.tile([HID, w], bf16, tag="wt", name="wt")
                    nc.vector.tensor_add(wt[:], v_t[:], st["pnx"][:])
                    st["wt"] = wt

            def dve_tail(st):
                w = st["w"]
                cur, ob = st["cur"], st["ob"]
                n_t, t_t = st["n"], st["tt"]
                u_t = wpool.tile([HID, w], bf16, tag="u", name="u")
                nc.vector.tensor_sub(u_t[:], n_t[:], t_t[:])
                g_t = wpool.tile([HID, w], bf16, tag="g", name="g")
                nc.vector.tensor_mul(g_t[:], st["zp"], u_t[:])
                nc.vector.tensor_add(cur[:, ob:ob + w], g_t[:], t_t[:])

            for t in range(ND):
                i0, i1, n, off = _DIAG[t]
                x_t = xpool.tile([INB, n * SB], bf16, tag="xt")
                nc.sync.dma_start(out=x_t[:], in_=xd[:, off:off + n * SB])
                x_tiles[t] = x_t

                chunks = [mk(t, c0, c1) for c0, c1 in parts_of(t)]
                # wider chunk first: its chain is the round's long pole
                chunks.sort(key=lambda s: -s["w"])
                c1s = chunks[0]
                c2s = chunks[1] if len(chunks) > 1 else None

                # Emission order follows dataflow; per-engine FIFO order is
                # what each engine sees:
                #  PE:  x(c1) x(c2) | h(c1) | h(c2,rn) I*v(c1) h(c2,z) I*v(c2)
                #  ACT: rz(c1) rz(c2) n(c1) n(c2)
                #  DVE: prep(c1) prep(c2) v(c1) tail(c1) v(c2) tail(c2)
                wtot = n * SB
                ch = 2 if len(chunks) > 1 else 1
                wmax = c1s["w"]
                pe_real = 4.17 * wtot + 1200.0
                dve_est = 600.0 * ch + 2.71 * wtot
                act_est = 480.0 * ch + 2.5 * wtot
                chain_est = 1500.0 + 5.52 * wmax
                round_est = max(dve_est, act_est, chain_est, pe_real)
                ndum = max(0, min(10, int((round_est - pe_real) / 213)))
                k1 = min(5, ndum) if ch == 2 else min(2, ndum)
                pe_x(c1s)
                if c2s is not None:
                    pe_x(c2s)
                pe_h(c1s, ("r", "n", "z"))
                if c2s is not None:
                    pe_h(c2s, ("r",))
                dve_prep(c1s)
                if c2s is not None:
                    dve_prep(c2s)
                act_rz(c1s)
                dve_v(c1s)
                if c2s is None and not c1s["small"]:
                    pe_fill(c1s, min(2, ndum))
                if not c1s["small"]:
                    pe_iv(c1s)
                if c2s is not None:
                    pe_h(c2s, ("n", "z"))
                act_n(c1s)
                dve_tail(c1s)
                if c2s is not None:
                    act_rz(c2s)
                    dve_v(c2s)
                    pe_fill(c1s, k1)
                    if not c2s["small"]:
                        pe_iv(c2s)
                    act_n(c2s)
                    dve_tail(c2s)
                    pe_fill(c1s, ndum - k1)
                else:
                    pe_fill(c1s, ndum - min(2, ndum))

                # one output DMA per diagonal, issued from the gpsimd queue
                cur = hbufs[t % 3]
                nc.gpsimd.dma_start(
                    out=od[:, off:off + n * SB],
                    in_=cur[:, (i0 + 1) * SB:(i1 + 2) * SB])

    nc.finalize()
    return nc


def _host_prep(x, Wx, Wh, Wh2, b):
    """Build per-core input maps (8 cores = 4 dirs x 2 batch halves)."""
    import ml_dtypes
    bf16 = ml_dtypes.bfloat16
    xr = np.ascontiguousarray(np.transpose(x, (2, 3, 0, 1))[:G, :G])  # (G,G,B,IN)
    ident = np.eye(HID, dtype=np.float32).astype(bf16)
    in_maps = []
    for d, (fy, fx) in enumerate(_FLIPS):
        xg = xr[::-1] if fy else xr
        xg = xg[:, ::-1] if fx else xg
        # fold bias into a ones-row; negate the z-gate block so
        # z' = sigmoid(pz) directly
        wx_e = np.concatenate([Wx[d], b[d][None, :]], axis=0).astype(np.float32)
        wh_e = np.array(Wh[d], np.float32)
        wh2_e = np.array(Wh2[d], np.float32)
        for m in (wx_e, wh_e, wh2_e):
            m[:, HID:2 * HID] *= -1.0
        wx_b = np.ascontiguousarray(wx_e).astype(bf16)
        wh_b = np.ascontiguousarray(wh_e).astype(bf16)
        wh2_b = np.ascontiguousarray(wh2_e).astype(bf16)
        for half in range(2):
            b0 = half * SB
            xh = xg[:, :, b0:b0 + SB]          # (G,G,SB,IN)
            xdiag = np.empty((INB, TOTAL), bf16)
            xdiag[IN, :] = np.float32(1.0)
            for t in range(ND):
                i0, i1, n, off = _DIAG[t]
                ii = np.arange(i0, i1 + 1)
                blk = xh[ii, t - ii]           # (n, SB, IN)
                xdiag[:IN, off:off + n * SB] = blk.reshape(n * SB, IN).T
            in_maps.append({
                "xd": xdiag,
                "wx": wx_b,
                "wh": wh_b,
                "wh2": wh2_b,
                "ident": ident,
            })
    return in_maps


def _host_gather(results):
    out_map = np.ones((4, H_IMG, W_IMG, B, HID), np.float32)
    for d, (fy, fx) in enumerate(_FLIPS):
        o = np.empty((G, G, B, HID), np.float32)
        for half in range(2):
            od = np.asarray(results[d * 2 + half]["od"], np.float32)  # (HID, TOTAL)
            b0 = half * SB
            for t in range(ND):
                i0, i1, n, off = _DIAG[t]
                sl = od[:, off:off + n * SB]
                blk = sl.T.reshape(n, SB, HID)
                ii = np.arange(i0, i1 + 1)
                o[ii, t - ii, b0:b0 + SB] = blk
        o = o[::-1] if fy else o
        o = o[:, ::-1] if fx else o
        oy, ox = (1 if fy else 0), (1 if fx else 0)
        out_map[d, oy:oy + G, ox:ox + G] = o
    return np.ascontiguousarray(np.transpose(out_map, (3, 4, 0, 1, 2)))


def kernel(x, Wx, Wh, Wh2, b):
    from concourse.bass_utils import run_bass_kernel_spmd

    if "prog" not in _PROG_CACHE:
        _PROG_CACHE["prog"] = _build_program()
    nc = _PROG_CACHE["prog"]

    in_maps = _host_prep(
        np.asarray(x, np.float32), np.asarray(Wx, np.float32),
        np.asarray(Wh, np.float32), np.asarray(Wh2, np.float32),
        np.asarray(b, np.float32))
    trace = os.environ.get("MDRNN_TRACE", "0") == "1"
    res = run_bass_kernel_spmd(nc, in_maps, list(range(8)), trace=trace)
    out = _host_gather(res.results)
    if trace:
        kernel.last_exec_time_ns = res.exec_time_ns
        kernel.last_profile = res
    return out


# revision 15
# speedup vs baseline: 1.9096x; 1.0991x over previous
"""MDRNN2D (4-direction 2D GRU) Trainium2 Bass kernel — bf16 wavefront v2.

Sharding: 8 cores = 4 scan directions x 2 batch halves (16 each).
Each core runs a wavefront over the 125 anti-diagonals of its (flipped)
63x63 grid. Hidden state for a diagonal lives in SBUF as (128=hid,
cells*16) bf16; h_up / h_left of diag t are 16-col-shifted slices of
diag t-1's buffer (3-buffer ring).

Each diagonal is split at a FIXED global cell boundary (cell 32) into
chunks A (i<32) and B (i>=32).  Cell neighbors are always on the
previous diagonal, so A(t) and B(t) both depend only on round t-1 =>
the two chunk-streams are independent chains that pipeline against
each other on the engines.

v2 structural changes vs v1:
- All biases folded into the matmuls via a ones-row on x (K=65) and the
  bias row appended to Wx; z-gate weights pre-negated on the host so
  z' = sigmoid(pz) directly.  No ACT bias ports used.
- pr and pz share one PSUM tile (prz, [128, 2w]); on narrow chunks one
  merged sigmoid ACT produces r|z' in a single instruction.
- Wide chunks: the n-gate add (gx_n + r*png) is done by accumulating
  an identity matmul of v = r*png into the open pnx PSUM group; tanh
  then reads PSUM directly.  Kills the 1x-throughput PSUM-source DVE
  add.  Narrow chunks keep the DVE add (fewer cross-engine hops).
- Narrow chunks use fused scalar_tensor_tensor for the tail:
  mu = 0.5*s - n; g2 = z'*mu; h = 0.5*s - g2  (4 DVE ops vs 5).
- No dummy warm-up matmuls: real work (I*v) keeps PE's HAM warm.
- Wider chunk emitted first each round (fixes descending-side stall).
"""

import os

import numpy as np

B, IN, H_IMG, W_IMG, HID = 32, 64, 64, 64, 128
INB = IN + 1      # x rows + ones-row (bias fold)
G = 63            # computed grid is (H-1, W-1)
ND = 2 * G - 1    # number of anti-diagonals
SB = 16           # batch per core
STOT = G * G * SB
TOTAL = STOT
GB = 32           # fixed A/B chunk boundary in global cell index
SMALL_W = 256     # chunks narrower than this use the low-latency path

# (i0, i1, ncells, col_offset) per diagonal; cells of diag t are (i, t-i),
# i in [i0, i1], stored as SB consecutive columns per cell, i ascending.
_DIAG = []
_off = 0
for _t in range(ND):
    _i0, _i1 = max(0, _t - (G - 1)), min(_t, G - 1)
    _n = _i1 - _i0 + 1
    _DIAG.append((_i0, _i1, _n, _off))
    _off += _n * SB
assert _off == STOT

_FLIPS = [(False, False), (True, False), (False, True), (True, True)]

_PROG_CACHE = {}


def _build_program():
    import concourse.mybir as mybir
    import concourse.tile as tile
    from concourse import bacc

    f32 = mybir.dt.float32
    bf16 = mybir.dt.bfloat16
    AF = mybir.ActivationFunctionType
    OP = mybir.AluOpType

    nc = bacc.Bacc()
    xd = nc.declare_dram_parameter("xd", [INB, TOTAL], bf16, isOutput=False)
    wxp = nc.declare_dram_parameter("wx", [INB, 3 * HID], bf16, isOutput=False)
    whp = nc.declare_dram_parameter("wh", [HID, 3 * HID], bf16, isOutput=False)
    wh2p = nc.declare_dram_parameter("wh2", [HID, 3 * HID], bf16, isOutput=False)
    idp = nc.declare_dram_parameter("ident", [HID, HID], bf16, isOutput=False)
    od = nc.declare_dram_parameter("od", [HID, TOTAL], bf16, isOutput=True)

    with tile.TileContext(nc) as tc:
        with (
            tc.tile_pool(name="const", bufs=1) as cpool,
            tc.tile_pool(name="hbuf", bufs=1) as hpool,
            tc.tile_pool(name="xin", bufs=6) as xpool,
            tc.tile_pool(name="ps", bufs=2, space="PSUM") as ppool,
            tc.tile_pool(name="work", bufs=3) as wpool,
        ):
            wx_t = cpool.tile([INB, 3 * HID], bf16, tag="wx")
            wh_t = cpool.tile([HID, 3 * HID], bf16, tag="wh")
            wh2_t = cpool.tile([HID, 3 * HID], bf16, tag="wh2")
            id_t = cpool.tile([HID, HID], bf16, tag="ident")
            nc.sync.dma_start(out=wx_t[:], in_=wxp[:])
            nc.sync.dma_start(out=wh_t[:], in_=whp[:])
            nc.sync.dma_start(out=wh2_t[:], in_=wh2p[:])
            nc.sync.dma_start(out=id_t[:], in_=idp[:])

            hbufs = []
            for k in range(3):
                hb = hpool.tile([HID, (G + 2) * SB], bf16, tag=f"h{k}",
                                name=f"hring{k}")
                hbufs.append(hb)
                nc.vector.memset(hb[:], 0.0)

            wxg = [wx_t[:, g * HID:(g + 1) * HID] for g in range(3)]
            whg = [wh_t[:, g * HID:(g + 1) * HID] for g in range(3)]
            wh2g = [wh2_t[:, g * HID:(g + 1) * HID] for g in range(3)]

            x_tiles = {}

            def parts_of(t):
                i0, i1, n, _ = _DIAG[t]
                if i1 < GB:
                    return [(0, n)]
                if i0 >= GB:
                    return [(0, n)]
                return [(0, GB - i0), (GB - i0, n)]

            # Per-chunk state dicts built in phases so each engine's FIFO
            # order can interleave the chunks of one round.
            def mk(t, c0, c1):
                i0, i1, n, off = _DIAG[t]
                w = (c1 - c0) * SB
                st = {
                    "t": t, "c0": c0, "c1": c1, "w": w,
                    "small": w < SMALL_W,
                    "cur": hbufs[t % 3], "prev": hbufs[(t - 1) % 3],
                    "xs": c0 * SB,
                    "ob": (i0 + 1 + c0) * SB,
                    "up": (i0 + c0) * SB,
                    "lf": (i0 + 1 + c0) * SB,
                }
                st["prz"] = ppool.tile([HID, 2, 512], f32, tag="prz", name="prz")
                st["png"] = ppool.tile([HID, w], f32, tag="png", name="png")
                st["pnx"] = ppool.tile([HID, w], f32, tag="pnx", name="pnx")
                return st

            def pe_x(st):
                # x-part matmuls (incl. ones-row bias): open all 3 groups
                w, xs = st["w"], st["xs"]
                xin = x_tiles[st["t"]][:, xs:xs + w]
                prz, pnx = st["prz"], st["pnx"]
                # wide path: pnx group stays open for the I*v accumulate
                nc.tensor.matmul(pnx[:], wxg[2], xin, start=True,
                                 stop=st["small"])
                nc.tensor.matmul(prz[:, 0, 0:w], wxg[0], xin, start=True, stop=False)
                nc.tensor.matmul(prz[:, 1, 0:w], wxg[1], xin, start=True, stop=False)

            def pe_h(st, gates):
                w = st["w"]
                prev = st["prev"]
                h_up = prev[:, st["up"]:st["up"] + w]
                h_left = prev[:, st["lf"]:st["lf"] + w]
                prz, png = st["prz"], st["png"]
                for gsel in gates:
                    if gsel == "r":
                        nc.tensor.matmul(prz[:, 0, 0:w], whg[0], h_up,
                                         start=False, stop=False)
                        nc.tensor.matmul(prz[:, 0, 0:w], wh2g[0], h_left,
                                         start=False, stop=True)
                    elif gsel == "n":
                        nc.tensor.matmul(png[:], whg[2], h_up,
                                         start=True, stop=False)
                        nc.tensor.matmul(png[:], wh2g[2], h_left,
                                         start=False, stop=True)
                    else:
                        nc.tensor.matmul(prz[:, 1, 0:w], whg[1], h_up,
                                         start=False, stop=False)
                        nc.tensor.matmul(prz[:, 1, 0:w], wh2g[1], h_left,
                                         start=False, stop=True)

            def pe_fill(st, k):
                # HAM-warmth filler: the PE clock-gate re-throttles to 1.2GHz
                # whenever its activity duty cycle drops, so the engine-idle
                # windows during each round's elementwise tail must be packed
                # with throwaway matmuls (into an already-consumed psum bank).
                for _ in range(k):
                    nc.tensor.matmul(st["prz"][:, 0, 0:512], id_t[:],
                                     st["prev"][:, 0:512], start=True,
                                     stop=True, skip_group_check=True)

            def pe_iv(st):
                # accumulate I*v into the still-open pnx group (wide path)
                nc.tensor.matmul(st["pnx"][:], id_t[:], st["v"][:],
                                 start=False, stop=True)

            def act_rz(st):
                w = st["w"]
                r_t = wpool.tile([HID, w], bf16, tag="r", name="r")
                nc.scalar.activation(r_t[:], st["prz"][:, 0, 0:w], AF.Sigmoid)
                st["r"] = r_t[:]
                zp_t = wpool.tile([HID, w], bf16, tag="zp", name="zp")
                nc.scalar.activation(zp_t[:], st["prz"][:, 1, 0:w],
                                     AF.Sigmoid)
                st["zp"] = zp_t[:]

            def act_n(st):
                w = st["w"]
                n_t = wpool.tile([HID, w], bf16, tag="n", name="n")
                if st["small"]:
                    nc.scalar.activation(n_t[:], st["wt"][:], AF.Tanh)
                else:
                    nc.scalar.activation(n_t[:], st["pnx"][:], AF.Tanh)
                st["n"] = n_t

            def dve_prep(st):
                # s = h_up + h_left (t = 0.5*s only on the wide path)
                w = st["w"]
                prev = st["prev"]
                h_up = prev[:, st["up"]:st["up"] + w]
                h_left = prev[:, st["lf"]:st["lf"] + w]
                s_t = wpool.tile([HID, w], bf16, tag="s", name="s")
                nc.vector.tensor_add(s_t[:], h_up, h_left)
                st["s"] = s_t
                t_t = wpool.tile([HID, w], bf16, tag="tt", name="tt")
                nc.vector.tensor_scalar_mul(t_t[:], s_t[:], 0.5)
                st["tt"] = t_t

            def dve_v(st):
                w = st["w"]
                v_t = wpool.tile([HID, w], bf16, tag="v", name="v")
                nc.vector.tensor_mul(v_t[:], st["r"], st["png"][:])
                st["v"] = v_t
                if st["small"]:
                    wt = wpool.tile([HID, w], bf16, tag="wt", name="wt")
                    nc.vector.tensor_add(wt[:], v_t[:], st["pnx"][:])
                    st["wt"] = wt

            def dve_tail(st):
                w = st["w"]
                cur, ob = st["cur"], st["ob"]
                n_t, t_t = st["n"], st["tt"]
                u_t = wpool.tile([HID, w], bf16, tag="u", name="u")
                nc.vector.tensor_sub(u_t[:], n_t[:], t_t[:])
                g_t = wpool.tile([HID, w], bf16, tag="g", name="g")
                nc.vector.tensor_mul(g_t[:], st["zp"], u_t[:])
                nc.vector.tensor_add(cur[:, ob:ob + w], g_t[:], t_t[:])

            state_cache = {}

            def get_chunks(t):
                # chunk states are created lazily so round t can emit round
                # t+1's x-matmuls (lookahead) into its own PE gap
                if t not in state_cache:
                    i0, i1, n, off = _DIAG[t]
                    x_t = xpool.tile([INB, n * SB], bf16, tag="xt", name="xt")
                    nc.sync.dma_start(out=x_t[:], in_=xd[:, off:off + n * SB])
                    x_tiles[t] = x_t
                    cl = [mk(t, c0, c1) for c0, c1 in parts_of(t)]
                    cl.sort(key=lambda s: -s["w"])
                    state_cache[t] = cl
                return state_cache[t]

            def pe_x_round(t):
                if t >= ND:
                    return
                for st in get_chunks(t):
                    pe_x(st)

            pe_x_round(0)
            for t in range(ND):
                i0, i1, n, off = _DIAG[t]
                chunks = get_chunks(t)
                c1s = chunks[0]
                c2s = chunks[1] if len(chunks) > 1 else None

                # Emission order follows dataflow; per-engine FIFO order is
                # what each engine sees:
                #  PE:  x(c1) x(c2) | h(c1) | h(c2,rn) I*v(c1) h(c2,z) I*v(c2)
                #  ACT: rz(c1) rz(c2) n(c1) n(c2)
                #  DVE: prep(c1) prep(c2) v(c1) tail(c1) v(c2) tail(c2)
                wtot = n * SB
                ndum = 4 if wtot >= 768 else max(2, min(9, int(
                    (3400 - 3.75 * wtot) / 213)))
                k1 = min(3, ndum) if c2s is not None else 0
                pe_h(c1s, ("r", "n", "z"))
                if c2s is not None:
                    pe_h(c2s, ("r",))
                dve_prep(c1s)
                if c2s is not None:
                    dve_prep(c2s)
                act_rz(c1s)
                dve_v(c1s)
                if c2s is None:
                    # single chunk: next round's x-matmuls fill the wait
                    # for v without delaying anything (I*v comes right after)
                    pe_x_round(t + 1)
                if not c1s["small"]:
                    pe_iv(c1s)
                if c2s is not None:
                    pe_h(c2s, ("n", "z"))
                act_n(c1s)
                dve_tail(c1s)
                if c2s is not None:
                    act_rz(c2s)
                    dve_v(c2s)
                    pe_x_round(t + 1)
                    pe_fill(c1s, k1)
                    if not c2s["small"]:
                        pe_iv(c2s)
                    act_n(c2s)
                    dve_tail(c2s)
                    pe_fill(c1s, ndum - k1)
                else:
                    pe_fill(c1s, ndum)

                # one output DMA per diagonal, issued from the gpsimd queue
                cur = hbufs[t % 3]
                nc.gpsimd.dma_start(
                    out=od[:, off:off + n * SB],
                    in_=cur[:, (i0 + 1) * SB:(i1 + 2) * SB])

    nc.finalize()
    return nc


def _host_prep(x, Wx, Wh, Wh2, b):
    """Build per-core input maps (8 cores = 4 dirs x 2 batch halves)."""
    import ml_dtypes
    bf16 = ml_dtypes.bfloat16
    xr = np.ascontiguousarray(np.transpose(x, (2, 3, 0, 1))[:G, :G])  # (G,G,B,IN)
    ident = np.eye(HID, dtype=np.float32).astype(bf16)
    in_maps = []
    for d, (fy, fx) in enumerate(_FLIPS):
        xg = xr[::-1] if fy else xr
        xg = xg[:, ::-1] if fx else xg
        # fold bias into a ones-row; negate the z-gate block so
        # z' = sigmoid(pz) directly
        wx_e = np.concatenate([Wx[d], b[d][None, :]], axis=0).astype(np.float32)
        wh_e = np.array(Wh[d], np.float32)
        wh2_e = np.array(Wh2[d], np.float32)
        for m in (wx_e, wh_e, wh2_e):
            m[:, HID:2 * HID] *= -1.0
        wx_b = np.ascontiguousarray(wx_e).astype(bf16)
        wh_b = np.ascontiguousarray(wh_e).astype(bf16)
        wh2_b = np.ascontiguousarray(wh2_e).astype(bf16)
        for half in range(2):
            b0 = half * SB
            xh = xg[:, :, b0:b0 + SB]          # (G,G,SB,IN)
            xdiag = np.empty((INB, TOTAL), bf16)
            xdiag[IN, :] = np.float32(1.0)
            for t in range(ND):
                i0, i1, n, off = _DIAG[t]
                ii = np.arange(i0, i1 + 1)
                blk = xh[ii, t - ii]           # (n, SB, IN)
                xdiag[:IN, off:off + n * SB] = blk.reshape(n * SB, IN).T
            in_maps.append({
                "xd": xdiag,
                "wx": wx_b,
                "wh": wh_b,
                "wh2": wh2_b,
                "ident": ident,
            })
    return in_maps


def _host_gather(results):
    out_map = np.ones((4, H_IMG, W_IMG, B, HID), np.float32)
    for d, (fy, fx) in enumerate(_FLIPS):
        o = np.empty((G, G, B, HID), np.float32)
        for half in range(2):
            od = np.asarray(results[d * 2 + half]["od"], np.float32)  # (HID, TOTAL)
            b0 = half * SB
            for t in range(ND):
                i0, i1, n, off = _DIAG[t]
                sl = od[:, off:off + n * SB]
                blk = sl.T.reshape(n, SB, HID)
                ii = np.arange(i0, i1 + 1)
                o[ii, t - ii, b0:b0 + SB] = blk
        o = o[::-1] if fy else o
        o = o[:, ::-1] if fx else o
        oy, ox = (1 if fy else 0), (1 if fx else 0)
        out_map[d, oy:oy + G, ox:ox + G] = o
    return np.ascontiguousarray(np.transpose(out_map, (3, 4, 0, 1, 2)))


def kernel(x, Wx, Wh, Wh2, b):
    from concourse.bass_utils import run_bass_kernel_spmd

    if "prog" not in _PROG_CACHE:
        _PROG_CACHE["prog"] = _build_program()
    nc = _PROG_CACHE["prog"]

    in_maps = _host_prep(
        np.asarray(x, np.float32), np.asarray(Wx, np.float32),
        np.asarray(Wh, np.float32), np.asarray(Wh2, np.float32),
        np.asarray(b, np.float32))
    trace = os.environ.get("MDRNN_TRACE", "0") == "1"
    res = run_bass_kernel_spmd(nc, in_maps, list(range(8)), trace=trace)
    out = _host_gather(res.results)
    if trace:
        kernel.last_exec_time_ns = res.exec_time_ns
        kernel.last_profile = res
    return out


# revision 18
# speedup vs baseline: 2.0341x; 1.0652x over previous
"""MDRNN2D (4-direction 2D GRU) Trainium2 Bass kernel — bf16 wavefront v2.

Sharding: 8 cores = 4 scan directions x 2 batch halves (16 each).
Each core runs a wavefront over the 125 anti-diagonals of its (flipped)
63x63 grid. Hidden state for a diagonal lives in SBUF as (128=hid,
cells*16) bf16; h_up / h_left of diag t are 16-col-shifted slices of
diag t-1's buffer (3-buffer ring).

Each diagonal is split at a FIXED global cell boundary (cell 32) into
chunks A (i<32) and B (i>=32).  Cell neighbors are always on the
previous diagonal, so A(t) and B(t) both depend only on round t-1 =>
the two chunk-streams are independent chains that pipeline against
each other on the engines.

v2 structural changes vs v1:
- All biases folded into the matmuls via a ones-row on x (K=65) and the
  bias row appended to Wx; z-gate weights pre-negated on the host so
  z' = sigmoid(pz) directly.  No ACT bias ports used.
- pr and pz share one PSUM tile (prz, [128, 2w]); on narrow chunks one
  merged sigmoid ACT produces r|z' in a single instruction.
- Wide chunks: the n-gate add (gx_n + r*png) is done by accumulating
  an identity matmul of v = r*png into the open pnx PSUM group; tanh
  then reads PSUM directly.  Kills the 1x-throughput PSUM-source DVE
  add.  Narrow chunks keep the DVE add (fewer cross-engine hops).
- Narrow chunks use fused scalar_tensor_tensor for the tail:
  mu = 0.5*s - n; g2 = z'*mu; h = 0.5*s - g2  (4 DVE ops vs 5).
- No dummy warm-up matmuls: real work (I*v) keeps PE's HAM warm.
- Wider chunk emitted first each round (fixes descending-side stall).
"""

import os

import numpy as np

B, IN, H_IMG, W_IMG, HID = 32, 64, 64, 64, 128
INB = IN + 1      # x rows + ones-row (bias fold)
G = 63            # computed grid is (H-1, W-1)
ND = 2 * G - 1    # number of anti-diagonals
SB = 16           # batch per core
STOT = G * G * SB
TOTAL = STOT
GB = 32           # fixed A/B chunk boundary in global cell index
SMALL_W = 256     # chunks narrower than this use the low-latency path

# (i0, i1, ncells, col_offset) per diagonal; cells of diag t are (i, t-i),
# i in [i0, i1], stored as SB consecutive columns per cell, i ascending.
_DIAG = []
_off = 0
for _t in range(ND):
    _i0, _i1 = max(0, _t - (G - 1)), min(_t, G - 1)
    _n = _i1 - _i0 + 1
    _DIAG.append((_i0, _i1, _n, _off))
    _off += _n * SB
assert _off == STOT

_FLIPS = [(False, False), (True, False), (False, True), (True, True)]

_PROG_CACHE = {}


def _build_program():
    import concourse.mybir as mybir
    import concourse.tile as tile
    from concourse import bacc

    f32 = mybir.dt.float32
    bf16 = mybir.dt.bfloat16
    AF = mybir.ActivationFunctionType
    OP = mybir.AluOpType

    nc = bacc.Bacc()
    xd = nc.declare_dram_parameter("xd", [INB, TOTAL], bf16, isOutput=False)
    wxp = nc.declare_dram_parameter("wx", [INB, 3 * HID], bf16, isOutput=False)
    whp = nc.declare_dram_parameter("wh", [HID, 3 * HID], bf16, isOutput=False)
    wh2p = nc.declare_dram_parameter("wh2", [HID, 3 * HID], bf16, isOutput=False)
    idp = nc.declare_dram_parameter("ident", [HID, HID], bf16, isOutput=False)
    od = nc.declare_dram_parameter("od", [HID, TOTAL], bf16, isOutput=True)

    with tile.TileContext(nc) as tc:
        with (
            tc.tile_pool(name="const", bufs=1) as cpool,
            tc.tile_pool(name="hbuf", bufs=1) as hpool,
            tc.tile_pool(name="xin", bufs=6) as xpool,
            tc.tile_pool(name="ps", bufs=2, space="PSUM") as ppool,
            tc.tile_pool(name="work", bufs=3) as wpool,
        ):
            wx_t = cpool.tile([INB, 3 * HID], bf16, tag="wx")
            wh_t = cpool.tile([HID, 3 * HID], bf16, tag="wh")
            wh2_t = cpool.tile([HID, 3 * HID], bf16, tag="wh2")
            id_t = cpool.tile([HID, HID], bf16, tag="ident")
            nc.sync.dma_start(out=wx_t[:], in_=wxp[:])
            nc.sync.dma_start(out=wh_t[:], in_=whp[:])
            nc.sync.dma_start(out=wh2_t[:], in_=wh2p[:])
            nc.sync.dma_start(out=id_t[:], in_=idp[:])

            hbufs = []
            for k in range(3):
                hb = hpool.tile([HID, (G + 2) * SB], bf16, tag=f"h{k}",
                                name=f"hring{k}")
                hbufs.append(hb)
                nc.vector.memset(hb[:], 0.0)

            wxg = [wx_t[:, g * HID:(g + 1) * HID] for g in range(3)]
            whg = [wh_t[:, g * HID:(g + 1) * HID] for g in range(3)]
            wh2g = [wh2_t[:, g * HID:(g + 1) * HID] for g in range(3)]

            x_tiles = {}

            def parts_of(t):
                # one boundary, two chunks max; descending side shifts the
                # boundary to 40 (once A fits in a bank) so B stays narrow
                i0, i1, n, _ = _DIAG[t]
                gb = GB if i0 < 8 else GB_DSC
                cuts = [gb - i0] if i0 < gb <= i1 else []
                edges = [0] + cuts + [n]
                return list(zip(edges[:-1], edges[1:]))

            # Per-chunk state dicts built in phases so each engine's FIFO
            # order can interleave the chunks of one round.
            def mk(t, c0, c1):
                i0, i1, n, off = _DIAG[t]
                w = (c1 - c0) * SB
                st = {
                    "t": t, "c0": c0, "c1": c1, "w": w,
                    "small": w < SMALL_W,
                    "cur": hbufs[t % 3], "prev": hbufs[(t - 1) % 3],
                    "xs": c0 * SB,
                    "ob": (i0 + 1 + c0) * SB,
                    "up": (i0 + c0) * SB,
                    "lf": (i0 + 1 + c0) * SB,
                }
                st["prz"] = ppool.tile([HID, 2, 512], f32, tag="prz", name="prz")
                st["png"] = ppool.tile([HID, w], f32, tag="png", name="png")
                st["pnx"] = ppool.tile([HID, w], f32, tag="pnx", name="pnx")
                return st

            def pe_x(st):
                # x-part matmuls (incl. ones-row bias): open all 3 groups
                w, xs = st["w"], st["xs"]
                xin = x_tiles[st["t"]][:, xs:xs + w]
                prz, pnx = st["prz"], st["pnx"]
                # wide path: pnx group stays open for the I*v accumulate
                nc.tensor.matmul(pnx[:], wxg[2], xin, start=True,
                                 stop=st["small"])
                nc.tensor.matmul(prz[:, 0, 0:w], wxg[0], xin, start=True, stop=False)
                nc.tensor.matmul(prz[:, 1, 0:w], wxg[1], xin, start=True, stop=False)

            def pe_h(st, gates):
                w = st["w"]
                prev = st["prev"]
                h_up = prev[:, st["up"]:st["up"] + w]
                h_left = prev[:, st["lf"]:st["lf"] + w]
                prz, png = st["prz"], st["png"]
                for gsel in gates:
                    if gsel == "r":
                        nc.tensor.matmul(prz[:, 0, 0:w], whg[0], h_up,
                                         start=False, stop=False)
                        nc.tensor.matmul(prz[:, 0, 0:w], wh2g[0], h_left,
                                         start=False, stop=True)
                    elif gsel == "n":
                        nc.tensor.matmul(png[:], whg[2], h_up,
                                         start=True, stop=False)
                        nc.tensor.matmul(png[:], wh2g[2], h_left,
                                         start=False, stop=True)
                    else:
                        nc.tensor.matmul(prz[:, 1, 0:w], whg[1], h_up,
                                         start=False, stop=False)
                        nc.tensor.matmul(prz[:, 1, 0:w], wh2g[1], h_left,
                                         start=False, stop=True)

            def pe_fill(st, k):
                # HAM-warmth filler: the PE clock-gate re-throttles to 1.2GHz
                # whenever its activity duty cycle drops, so the engine-idle
                # windows during each round's elementwise tail must be packed
                # with throwaway matmuls (into an already-consumed psum bank).
                for _ in range(k):
                    nc.tensor.matmul(st["prz"][:, 0, 0:512], id_t[:],
                                     st["prev"][:, 0:512], start=True,
                                     stop=True, skip_group_check=True)

            def pe_iv(st):
                # accumulate I*v into the still-open pnx group (wide path)
                nc.tensor.matmul(st["pnx"][:], id_t[:], st["v"][:],
                                 start=False, stop=True)

            def act_rz(st):
                w = st["w"]
                r_t = wpool.tile([HID, w], bf16, tag="r", name="r")
                nc.scalar.activation(r_t[:], st["prz"][:, 0, 0:w], AF.Sigmoid)
                st["r"] = r_t[:]
                zp_t = wpool.tile([HID, w], bf16, tag="zp", name="zp")
                nc.scalar.activation(zp_t[:], st["prz"][:, 1, 0:w],
                                     AF.Sigmoid)
                st["zp"] = zp_t[:]

            def act_n(st):
                w = st["w"]
                n_t = wpool.tile([HID, w], bf16, tag="n", name="n")
                if st["small"]:
                    nc.scalar.activation(n_t[:], st["wt"][:], AF.Tanh)
                else:
                    nc.scalar.activation(n_t[:], st["pnx"][:], AF.Tanh)
                st["n"] = n_t

            def dve_prep(st):
                # s = h_up + h_left (t = 0.5*s only on the wide path)
                w = st["w"]
                prev = st["prev"]
                h_up = prev[:, st["up"]:st["up"] + w]
                h_left = prev[:, st["lf"]:st["lf"] + w]
                s_t = wpool.tile([HID, w], bf16, tag="s", name="s")
                nc.vector.tensor_add(s_t[:], h_up, h_left)
                st["s"] = s_t
                t_t = wpool.tile([HID, w], bf16, tag="tt", name="tt")
                nc.vector.tensor_scalar_mul(t_t[:], s_t[:], 0.5)
                st["tt"] = t_t

            def dve_v(st):
                w = st["w"]
                v_t = wpool.tile([HID, w], bf16, tag="v", name="v")
                nc.vector.tensor_mul(v_t[:], st["r"], st["png"][:])
                st["v"] = v_t
                if st["small"]:
                    wt = wpool.tile([HID, w], bf16, tag="wt", name="wt")
                    nc.vector.tensor_add(wt[:], v_t[:], st["pnx"][:])
                    st["wt"] = wt

            def dve_tail(st):
                w = st["w"]
                cur, ob = st["cur"], st["ob"]
                n_t, t_t = st["n"], st["tt"]
                u_t = wpool.tile([HID, w], bf16, tag="u", name="u")
                nc.vector.tensor_sub(u_t[:], n_t[:], t_t[:])
                g_t = wpool.tile([HID, w], bf16, tag="g", name="g")
                nc.vector.tensor_mul(g_t[:], st["zp"], u_t[:])
                nc.vector.tensor_add(cur[:, ob:ob + w], g_t[:], t_t[:])

            state_cache = {}

            def get_chunks(t):
                # chunk states are created lazily so round t can emit round
                # t+1's x-matmuls (lookahead) into its own PE gap
                if t not in state_cache:
                    i0, i1, n, off = _DIAG[t]
                    x_t = xpool.tile([INB, n * SB], bf16, tag="xt", name="xt")
                    nc.sync.dma_start(out=x_t[:], in_=xd[:, off:off + n * SB])
                    x_tiles[t] = x_t
                    state_cache[t] = [mk(t, c0, c1) for c0, c1 in parts_of(t)]
                return state_cache[t]

            def pe_x_round(t):
                if t >= ND:
                    return
                for st in get_chunks(t):
                    pe_x(st)

            pe_x_round(0)
            for t in range(ND):
                i0, i1, n, off = _DIAG[t]
                chunks = get_chunks(t)
                c1s = chunks[0]
                c2s = chunks[1] if len(chunks) > 1 else None

                # Emission order follows dataflow; per-engine FIFO order is
                # what each engine sees:
                #  PE:  x(c1) x(c2) | h(c1) | h(c2,rn) I*v(c1) h(c2,z) I*v(c2)
                #  ACT: rz(c1) rz(c2) n(c1) n(c2)
                #  DVE: prep(c1) prep(c2) v(c1) tail(c1) v(c2) tail(c2)
                wtot = n * SB
                ch = len(chunks)
                wmax = max(c["w"] for c in chunks)
                pe_real = 4.17 * wtot + 800.0
                dve_est = 941.0 * ch + 3.37 * wtot
                chain_est = 1500.0 + 5.52 * wmax
                bound = max(chain_est, dve_est, 1.12 * pe_real)
                ndum = max(1, min(9, int((bound - pe_real) / 213)))
                k1 = min(3, ndum) if c2s is not None else 0
                nch = len(chunks)
                pe_h(chunks[0], ("r", "n", "z"))
                if nch > 1:
                    pe_h(chunks[1], ("r",))
                for cs in chunks:
                    dve_prep(cs)
                for j, cs in enumerate(chunks):
                    act_rz(cs)
                    dve_v(cs)
                    last = j == nch - 1
                    if last:
                        # next round's x-matmuls + fills pack the tail window
                        pe_x_round(t + 1)
                        pe_fill(chunks[0], k1)
                    if not cs["small"]:
                        pe_iv(cs)
                    if not last:
                        pe_h(chunks[j + 1], ("n", "z"))
                        if j + 2 < nch:
                            pe_h(chunks[j + 2], ("r",))
                    act_n(cs)
                    dve_tail(cs)
                pe_fill(chunks[0], ndum - k1)

                # one output DMA per diagonal, issued from the gpsimd queue
                cur = hbufs[t % 3]
                nc.gpsimd.dma_start(
                    out=od[:, off:off + n * SB],
                    in_=cur[:, (i0 + 1) * SB:(i1 + 2) * SB])

    nc.finalize()
    return nc


def _host_prep(x, Wx, Wh, Wh2, b):
    """Build per-core input maps (8 cores = 4 dirs x 2 batch halves)."""
    import ml_dtypes
    bf16 = ml_dtypes.bfloat16
    xr = np.ascontiguousarray(np.transpose(x, (2, 3, 0, 1))[:G, :G])  # (G,G,B,IN)
    ident = np.eye(HID, dtype=np.float32).astype(bf16)
    in_maps = []
    for d, (fy, fx) in enumerate(_FLIPS):
        xg = xr[::-1] if fy else xr
        xg = xg[:, ::-1] if fx else xg
        # fold bias into a ones-row; negate the z-gate block so
        # z' = sigmoid(pz) directly
        wx_e = np.concatenate([Wx[d], b[d][None, :]], axis=0).astype(np.float32)
        wh_e = np.array(Wh[d], np.float32)
        wh2_e = np.array(Wh2[d], np.float32)
        for m in (wx_e, wh_e, wh2_e):
            m[:, HID:2 * HID] *= -1.0
        wx_b = np.ascontiguousarray(wx_e).astype(bf16)
        wh_b = np.ascontiguousarray(wh_e).astype(bf16)
        wh2_b = np.ascontiguousarray(wh2_e).astype(bf16)
        for half in range(2):
            b0 = half * SB
            xh = xg[:, :, b0:b0 + SB]          # (G,G,SB,IN)
            xdiag = np.empty((INB, TOTAL), bf16)
            xdiag[IN, :] = np.float32(1.0)
            for t in range(ND):
                i0, i1, n, off = _DIAG[t]
                ii = np.arange(i0, i1 + 1)
                blk = xh[ii, t - ii]           # (n, SB, IN)
                xdiag[:IN, off:off + n * SB] = blk.reshape(n * SB, IN).T
            in_maps.append({
                "xd": xdiag,
                "wx": wx_b,
                "wh": wh_b,
                "wh2": wh2_b,
                "ident": ident,
            })
    return in_maps


def _host_gather(results):
    out_map = np.ones((4, H_IMG, W_IMG, B, HID), np.float32)
    for d, (fy, fx) in enumerate(_FLIPS):
        o = np.empty((G, G, B, HID), np.float32)
        for half in range(2):
            od = np.asarray(results[d * 2 + half]["od"], np.float32)  # (HID, TOTAL)
            b0 = half * SB
            for t in range(ND):
                i0, i1, n, off = _DIAG[t]
                sl = od[:, off:off + n * SB]
                blk = sl.T.reshape(n, SB, HID)
                ii = np.arange(i0, i1 + 1)
                o[ii, t - ii, b0:b0 + SB] = blk
        o = o[::-1] if fy else o
        o = o[:, ::-1] if fx else o
        oy, ox = (1 if fy else 0), (1 if fx else 0)
        out_map[d, oy:oy + G, ox:ox + G] = o
    return np.ascontiguousarray(np.transpose(out_map, (3, 4, 0, 1, 2)))


def kernel(x, Wx, Wh, Wh2, b):
    from concourse.bass_utils import run_bass_kernel_spmd

    if "prog" not in _PROG_CACHE:
        _PROG_CACHE["prog"] = _build_program()
    nc = _PROG_CACHE["prog"]

    in_maps = _host_prep(
        np.asarray(x, np.float32), np.asarray(Wx, np.float32),
        np.asarray(Wh, np.float32), np.asarray(Wh2, np.float32),
        np.asarray(b, np.float32))
    trace = os.environ.get("MDRNN_TRACE", "0") == "1"
    res = run_bass_kernel_spmd(nc, in_maps, list(range(8)), trace=trace)
    out = _host_gather(res.results)
    if trace:
        kernel.last_exec_time_ns = res.exec_time_ns
        kernel.last_profile = res
    return out
